# revision 19
# baseline (speedup 1.0000x reference)
"""AtomicConv (gnn_message_passing) distributed Trainium2 kernel.

Strategy (hardcoded for N=100000 nodes, E=1600000 edges, K=8, T=4, 8 cores):
  - Shard edges by destination-node range: core c owns nodes
    [c*12500, (c+1)*12500) and every edge whose dst falls in that range.
    This removes the big (N,T,K) all-reduce entirely; only 16 floats of
    BatchNorm statistics are all-reduced.
  - Host does index-only layout: per core, nodes are relabelled in
    descending-degree order and each node's edges are packed into 4-slot
    "tier" rows (tier j holds edges [4j, 4j+4) of a node).  Tier arrays are
    prefix-aligned (degree sort), so per-tier partial sums accumulate into
    the same PSUM cells.
  - Device tile layout: partition p = n32*4 + s (32 nodes x 4 slots),
    free axis = block columns.  The segment-sum over slots is a TensorE
    matmul with a constant block-diagonal ones matrix (lhsT, loaded once);
    tiers accumulate in PSUM (start/stop flags, explicit tile_position).
  - ScalarE computes the radial basis exp(-gam*(d-mu)^2) with two
    activation ops per k (Square with bias=-mu_k, Exp with scale=-gam_k).
    The cosine-cutoff term is k-independent (uniform cutoffs) and costs one
    Sin + two small DVE ops per chunk.  Class gating (feat[src] ==
    features_to_use[t]) is fused with the cutoff into 4 bf16 masks, and 4
    masked multiplies produce the matmul rhs.
  - BatchNorm: per-core sum/sumsq stats from PSUM, partition_all_reduce,
    a 16-float AllReduce across the 8 cores, then the final scale/shift is
    fused into the PSUM->SBUF evacuation (one ACT op per psum bank).
"""

import sys, os

for _p in ("/opt/trn_rl_repo", "/root/.axon_site/_ro/trn_rl_repo"):
    if os.path.isdir(_p) and _p not in sys.path:
        sys.path.insert(0, _p)

import numpy as np

N = 100000
E = 1600000
K = 8
T = 4
NCORES = 8
NPC = N // NCORES          # 12500 nodes per core
EPS = 1e-5

SLOTS = 4                  # slots per tier row
BLK = 32                   # nodes per block (SLOTS*BLK = 128 partitions)
G64 = 64                   # blocks per psum stack
NSTACK = 4                 # stacks per psum region (4 x 32 out partitions)
REGION_BLOCKS = NSTACK * G64   # 256 blocks per psum region
CHUNK = 256                # block-columns per compute chunk
FS_PAD = -7777.0

_COMPILED = {}
LAST_RESULT = None
SKIP = set()   # bisect: 'pbcast','iota','mm','stats','preduce','cc','evac'


def _build_layout(feat, distances, src, dst, rbf_means):
    """Host-side index-only layout. Returns per-core dts/fss arrays plus
    decode info. Pure permutation/padding of input values."""
    dst = np.asarray(dst).astype(np.int64)
    src = np.asarray(src).astype(np.int64)
    dist = np.asarray(distances, dtype=np.float32).reshape(-1)
    featv = np.asarray(feat, dtype=np.float32).reshape(-1)
    muv = np.asarray(rbf_means, dtype=np.float32).reshape(-1)

    core = dst // NPC
    deg = np.zeros((NCORES, NPC), dtype=np.int64)
    np.add.at(deg, (core, dst % NPC), 1)

    maxdeg = int(deg.max())
    J = (maxdeg + SLOTS - 1) // SLOTS          # number of tiers
    NBW = []
    for j in range(J):
        if j == 0:
            nb = (NPC + BLK - 1) // BLK
            nb = ((nb + G64 - 1) // G64) * G64   # tier0: full psum stacks
        else:
            cnt = int((deg > SLOTS * j).sum(axis=1).max())
            nb = (cnt + BLK - 1) // BLK
            nb = ((nb + 7) // 8) * 8
        NBW.append(nb)
    offs = np.concatenate([[0], np.cumsum(NBW)]).astype(np.int64)
    GT = int(offs[-1])

    dts = np.zeros((NCORES, 128, 8, GT), dtype=np.float32)
    mus = np.zeros((NCORES, 128, GT), dtype=np.float32)
    fss = np.full((NCORES, 128, GT), FS_PAD, dtype=np.float32)
    dts_written = np.zeros((NCORES, 128, GT), dtype=bool)

    ranks = np.zeros((NCORES, NPC), dtype=np.int64)
    for c in range(NCORES):
        order = np.argsort(-deg[c], kind="stable")
        ranks[c, order] = np.arange(NPC)

    for c in range(NCORES):
        sel = np.nonzero(core == c)[0]
        dl = (dst[sel] % NPC)
        o2 = np.argsort(dl, kind="stable")
        sel = sel[o2]
        dl = dl[o2]
        first = np.concatenate([[0], np.cumsum(deg[c])[:-1]])
        kwithin = np.arange(sel.size) - first[dl]
        tier = kwithin // SLOTS
        slot = kwithin % SLOTS
        r = ranks[c, dl]
        p = (r % BLK) * SLOTS + slot
        col = offs[tier] + r // BLK
        # true reference semantics: edge e reads filter f=e//(E//K) applied
        # to the 8-run dist[8*(e%(E//K)) + k]  (the (K,E)->(E,K) reshape)
        base8 = (sel % (E // K)) * 8
        dts[c, p[:, None], np.arange(8)[None, :], col[:, None]] = \
            dist[base8[:, None] + np.arange(8)[None, :]]
        mus[c, p, col] = muv[sel // (E // K)]
        dts_written[c, p, col] = True
        fss[c, p, col] = featv[src[sel]]

    return dts, mus, fss, dts_written, NBW, offs, GT, ranks, J


def _build_graph(GT, NBW, offs, J, NP, gam_val, co_val, need_mask):
    """Build the Bass graph (same for all 8 cores)."""
    from concourse import bass, mybir, bacc, bass_isa
    import concourse.tile as tile

    f32 = mybir.dt.float32
    bf16 = mybir.dt.bfloat16
    ALU = mybir.AluOpType
    ACT = mybir.ActivationFunctionType

    NB0 = NBW[0]
    RG = (NB0 + REGION_BLOCKS - 1) // REGION_BLOCKS      # psum regions
    OUTW = T * RG * 8 * G64                              # (t, r, k, g) f32 cols
    CNT = float(N * K)

    nc = bacc.Bacc(None, target_bir_lowering=False, debug=False)

    dts_d = nc.declare_dram_parameter("dts", [128, 8 * GT], f32,
                                      isOutput=False)
    mus_d = nc.declare_dram_parameter("mus", [128, GT], f32, isOutput=False)
    fss_d = nc.declare_dram_parameter("fss", [128, GT], f32, isOutput=False)
    par_d = nc.declare_dram_parameter("par", [1, NP], f32, isOutput=False)
    out_d = nc.declare_dram_parameter("out", [128, OUTW], f32, isOutput=True)

    MU, GAM, CO, FTU, BNW, BNB = 0, K, 2 * K, 3 * K, 3 * K + T, 3 * K + 2 * T

    with tile.TileContext(nc) as tc:
        with (
            tc.tile_pool(name="const", bufs=1) as constp,
            tc.tile_pool(name="work", bufs=3) as work,
            tc.tile_pool(name="mtile", bufs=4) as mpool,
            tc.tile_pool(name="outp", bufs=1) as outp,
            tc.tile_pool(name="psum", bufs=1, space="PSUM") as psump,
            tc.tile_pool(name="dram", bufs=1, space="DRAM") as dramp,
        ):
            # ---- constants / params -------------------------------------
            par_row = constp.tile([1, NP], f32)
            nc.sync.dma_start(par_row[:], par_d[:, :])
            parb = constp.tile([128, NP], f32)
            if 'pbcast' in SKIP:
                nc.vector.memset(parb[:], 1.0)
            else:
                nc.gpsimd.partition_broadcast(parb[:], par_row[:])

            negmu = constp.tile([128, K], f32)
            nc.vector.tensor_scalar_mul(negmu[:], parb[:, MU:MU + K], -1.0)
            neggam = constp.tile([128, K], f32)
            nc.vector.tensor_scalar_mul(neggam[:], parb[:, GAM:GAM + K], -1.0)
            pioc = constp.tile([128, 1], f32)
            nc.vector.reciprocal(pioc[:], parb[:, CO:CO + 1])
            nc.vector.tensor_scalar_mul(pioc[:], pioc[:], float(np.pi))
            halfpi = constp.tile([128, 1], f32)
            nc.vector.memset(halfpi[:], float(np.pi / 2))
            zcol = constp.tile([128, 1], f32)
            nc.vector.memset(zcol[:], 0.0)

            # block-diagonal ones [128, 32] bf16: lhsT[c, m] = (c//4 == m)
            bdiag = constp.tile([128, BLK], bf16)
            if 'iota' in SKIP:
                nc.vector.memset(bdiag[:], 1.0)
            else:
                pid = constp.tile([128, 1], mybir.dt.int32)
                nc.gpsimd.iota(pid[:], [[0, 1]], channel_multiplier=1)
                nboff = constp.tile([128, 1], mybir.dt.int32)
                nc.vector.tensor_scalar(nboff[:], pid[:], 2, None,
                                        op0=ALU.logical_shift_right)
                nboff_f = constp.tile([128, 1], f32)
                nc.vector.tensor_copy(nboff_f[:], nboff[:])
                iot32 = constp.tile([128, BLK], mybir.dt.int32)
                nc.gpsimd.iota(iot32[:], [[1, BLK]], channel_multiplier=0)
                iot32_f = constp.tile([128, BLK], f32)
                nc.vector.tensor_copy(iot32_f[:], iot32[:])
                nc.vector.tensor_tensor(
                    bdiag[:], nboff_f[:].to_broadcast([128, BLK]), iot32_f[:],
                    op=ALU.is_equal)

            # ---- psum accumulators: 4 t x RG regions --------------------
            psums = [[psump.tile([128, 8 * G64], f32, name=f"ps{_t}_{_r}",
                                 tag=f"ps{_t}_{_r}")
                      for _r in range(RG)]
                     for _t in range(T)]

            def tiers_for(r, su):
                base = r * REGION_BLOCKS + su * G64
                return [j for j in range(J) if NBW[j] > base]

            # ---- main compute loop --------------------------------------
            # flat chunk list, processed in super-chunks of SC so the ACT
            # Exp/Sin calls can be grouped (amortizes activation-table loads)
            from concourse.tile import add_dep_helper
            chunks = []
            for j in range(J):
                for r in range(RG):
                    lo = r * REGION_BLOCKS
                    hi = min((r + 1) * REGION_BLOCKS, NBW[j])
                    c0 = lo
                    while c0 < hi:
                        cw = min(CHUNK, hi - c0)
                        chunks.append((j, r, lo, c0, cw))
                        c0 += cw
            SC = 2
            d8v = dts_d[:, :].rearrange("p (k g) -> p k g", k=K)
            last_act = [None]

            def act_chain(ins):
                if last_act[0] is not None:
                    add_dep_helper(ins.ins, last_act[0].ins, sync=False,
                                   reason="ACT table grouping")
                last_act[0] = ins

            for s0 in range(0, len(chunks), SC):
                group = chunks[s0:s0 + SC]
                gtiles = []
                for (j, r, lo, c0, cw) in group:
                    col0 = int(offs[j] + c0)
                    d8_t = work.tile([128, K, CHUNK], f32, tag="d8_t",
                                     bufs=3)
                    mu_t = work.tile([128, CHUNK], f32, tag="mu_t",
                                     bufs=3)
                    fs_t = work.tile([128, CHUNK], f32, tag="fs_t",
                                     bufs=3)
                    nc.sync.dma_start(d8_t[:, :, :cw],
                                      d8v[:, :, col0:col0 + cw])
                    nc.sync.dma_start(mu_t[:, :cw], mus_d[:, col0:col0 + cw])
                    nc.sync.dma_start(fs_t[:, :cw], fss_d[:, col0:col0 + cw])
                    # u = d8 - mu_e ; u2 = u*u   (GPSIMD/Pool engine)
                    sq_t = work.tile([128, K, CHUNK], f32, tag="sq_t",
                                     bufs=2)
                    sq2_t = work.tile([128, K, CHUNK], f32, tag="sq2_t",
                                      bufs=2)
                    nc.vector.tensor_tensor(
                        sq_t[:, :, :cw], d8_t[:, :, :cw],
                        mu_t[:, :cw].unsqueeze(1).to_broadcast([128, K, cw]),
                        op=ALU.subtract)
                    nc.gpsimd.tensor_tensor(
                        sq2_t[:, :, :cw], sq_t[:, :, :cw],
                        sq_t[:, :, :cw], op=ALU.mult)
                    gtiles.append((d8_t, mu_t, fs_t, sq2_t))

                # grouped ACT: all Exp, then all Sin (table amortization)
                rbfs, szs = [], []
                for gi, (j, r, lo, c0, cw) in enumerate(group):
                    d8_t, mu_t, fs_t, sq2_t = gtiles[gi]
                    rbf_t = work.tile([128, K, CHUNK], bf16, tag="rbf_t",
                                      bufs=3)
                    ins = nc.scalar.activation(
                        rbf_t[:, :, :cw], sq2_t[:, :, :cw], ACT.Exp,
                        bias=zcol[:, 0:1], scale=float(-gam_val))
                    act_chain(ins)
                    rbfs.append(rbf_t)
                for gi, (j, r, lo, c0, cw) in enumerate(group):
                    d8_t = gtiles[gi][0]
                    sz_t = work.tile([128, K, CHUNK], bf16, tag="sz_t",
                                     bufs=3)
                    ins = nc.scalar.activation(
                        sz_t[:, :, :cw], d8_t[:, :, :cw], ACT.Sin,
                        bias=halfpi[:, 0:1],
                        scale=float(-np.pi / (2.0 * co_val)))
                    act_chain(ins)
                    szs.append(sz_t)

                for gi, (j, r, lo, c0, cw) in enumerate(group):
                    d8_t, mu_t, fs_t, sq2_t = gtiles[gi]
                    rbf_t, sz_t = rbfs[gi], szs[gi]
                    # sz2 = sz*sz (bf16, DVE 2x); w8 = rbf*sz2 (Pool)
                    sz2_t = work.tile([128, K, CHUNK], bf16, tag="sz2_t",
                                      bufs=2)
                    nc.vector.tensor_tensor(
                        sz2_t[:, :, :cw], sz_t[:, :, :cw], sz_t[:, :, :cw],
                        op=ALU.mult)
                    w8_t = work.tile([128, K, CHUNK], bf16, tag="w8_t",
                                     bufs=2)
                    nc.vector.tensor_tensor(
                        w8_t[:, :, :cw], rbf_t[:, :, :cw],
                        sz2_t[:, :, :cw], op=ALU.mult)
                    if need_mask:
                        mk_t = work.tile([128, K, CHUNK], bf16, tag="mk_t")
                        nc.vector.tensor_scalar(
                            mk_t[:, :, :cw], d8_t[:, :, :cw],
                            parb[:, CO:CO + 1], None, op0=ALU.is_lt)
                        nc.vector.tensor_tensor(
                            w8_t[:, :, :cw], w8_t[:, :, :cw],
                            mk_t[:, :, :cw], op=ALU.mult)

                    # class gates (pure indicators)
                    ctc = work.tile([128, T, CHUNK], bf16, tag="ctc",
                                    bufs=2)
                    for t in range(T):
                        nc.vector.tensor_scalar(
                            ctc[:, t, :cw], fs_t[:, :cw],
                            parb[:, FTU + t:FTU + t + 1], None,
                            op0=ALU.is_equal)

                    # masked messages + matmul per stack
                    for t in range(T):
                        m_t = mpool.tile([128, K, CHUNK], bf16, tag="m_t")
                        nc.vector.tensor_tensor(
                            m_t[:, :, :cw], w8_t[:, :, :cw],
                            ctc[:, t:t + 1, :cw].to_broadcast([128, K, cw]),
                            op=ALU.mult)
                        u0 = 0
                        while u0 < cw:
                            su = (c0 - lo + u0) // G64
                            uw = min(G64, cw - u0)
                            js = tiers_for(r, su)
                            nc.tensor.matmul(
                                psums[t][r][32 * su:32 * su + 32,
                                            :].rearrange(
                                    "p (k g) -> p k g", k=K)[
                                    :, :, (c0 - lo + u0) % G64:
                                    (c0 - lo + u0) % G64 + uw],
                                bdiag[:, :],
                                m_t[:, :, u0:u0 + uw],
                                start=(j == js[0]),
                                stop=(j == js[-1]),
                                tile_position=(0, 32 * su),
                            )
                            u0 += uw

            # ---- statistics from PSUM -----------------------------------
            vstk = [min(NSTACK,
                        (NB0 - r * REGION_BLOCKS + G64 - 1) // G64)
                    for r in range(RG)]
            xs_all = constp.tile([128, 2 * T * RG], f32)
            nc.vector.memset(xs_all[:], 0.0)
            scratch = work.tile([128, 8 * G64], f32, tag="scratch")
            for t in range(T):
                for r in range(RG):
                    vp = 32 * vstk[r]
                    nc.vector.tensor_reduce(
                        xs_all[0:vp, t * RG + r:t * RG + r + 1],
                        psums[t][r][0:vp, :],
                        axis=mybir.AxisListType.X, op=ALU.add)
                    if 'accum' in SKIP:
                        nc.vector.tensor_copy(scratch[0:vp, :],
                                              psums[t][r][0:vp, :])
                    else:
                        nc.scalar.activation(
                            scratch[0:vp, :], psums[t][r][0:vp, :], ACT.Square,
                            accum_out=xs_all[0:vp, T * RG + t * RG + r:
                                             T * RG + t * RG + r + 1])
            xs_red = constp.tile([128, 2 * T * RG], f32)
            if 'preduce' in SKIP:
                nc.vector.tensor_copy(xs_red[:], xs_all[:])
            else:
                nc.gpsimd.partition_all_reduce(
                    xs_red[:], xs_all[:], channels=128,
                    reduce_op=bass_isa.ReduceOp.add)

            # ---- tiny AllReduce of stats across 8 cores ------------------
            cc_in = dramp.tile([1, 2 * T * RG], f32)
            cc_out = dramp.tile([1, 2 * T * RG], f32)
            nc.sync.dma_start(cc_in[:], xs_red[0:1, :])
            if 'cc' in SKIP:
                nc.sync.dma_start(cc_out[:], cc_in[:])
            else:
                nc.gpsimd.collective_compute(
                    "AllReduce", ALU.add,
                    replica_groups=[list(range(NCORES))],
                    ins=[cc_in.opt()],
                    outs=[cc_out.opt()],
                )
            gstat = constp.tile([1, 2 * T * RG], f32)
            nc.sync.dma_start(gstat[:], cc_out[:])

            # ---- scale/shift math on [1, x] ------------------------------
            xs4 = constp.tile([1, T], f32)
            xsq4 = constp.tile([1, T], f32)
            if RG > 1:
                nc.vector.tensor_reduce(
                    xs4[:], gstat[:, 0:T * RG].rearrange(
                        "p (t r) -> p t r", r=RG),
                    axis=mybir.AxisListType.X, op=ALU.add)
                nc.vector.tensor_reduce(
                    xsq4[:], gstat[:, T * RG:2 * T * RG].rearrange(
                        "p (t r) -> p t r", r=RG),
                    axis=mybir.AxisListType.X, op=ALU.add)
            else:
                nc.vector.tensor_copy(xs4[:], gstat[:, 0:T])
                nc.vector.tensor_copy(xsq4[:], gstat[:, T:2 * T])

            meanp = constp.tile([1, T], f32)
            nc.vector.tensor_scalar_mul(meanp[:], xs4[:], 1.0 / CNT)
            varp = constp.tile([1, T], f32)
            nc.vector.tensor_scalar_mul(varp[:], xsq4[:], 1.0 / CNT)
            m2 = constp.tile([1, T], f32)
            nc.vector.tensor_tensor(m2[:], meanp[:], meanp[:], op=ALU.mult)
            nc.vector.tensor_tensor(varp[:], varp[:], m2[:], op=ALU.subtract)
            ctv = constp.tile([1, T], f32)
            nc.vector.tensor_scalar_mul(ctv[:], par_row[:, FTU:FTU + T], 1.0)
            ct2 = constp.tile([1, T], f32)
            nc.vector.tensor_tensor(ct2[:], ctv[:], ctv[:], op=ALU.mult)
            nc.vector.tensor_tensor(varp[:], varp[:], ct2[:], op=ALU.mult)
            nc.vector.tensor_scalar_add(varp[:], varp[:], EPS)
            sd = constp.tile([1, T], f32)
            nc.scalar.sqrt(sd[:], varp[:])
            rsd = constp.tile([1, T], f32)
            nc.vector.reciprocal(rsd[:], sd[:])

            ns_row = constp.tile([1, 2 * T], f32)
            nc.vector.tensor_tensor(ns_row[:, 0:T], ctv[:], rsd[:],
                                    op=ALU.mult)
            nc.vector.tensor_tensor(ns_row[:, 0:T], ns_row[:, 0:T],
                                    par_row[:, BNW:BNW + T], op=ALU.mult)
            nc.vector.tensor_tensor(ns_row[:, T:2 * T], meanp[:],
                                    ns_row[:, 0:T], op=ALU.mult)
            nc.vector.tensor_tensor(ns_row[:, T:2 * T],
                                    par_row[:, BNB:BNB + T],
                                    ns_row[:, T:2 * T], op=ALU.subtract)
            ns_bc = constp.tile([128, 2 * T], f32)
            if 'pbcast' in SKIP:
                nc.vector.memset(ns_bc[:], 1.0)
            else:
                nc.gpsimd.partition_broadcast(ns_bc[:], ns_row[:])

            # ---- fused evacuate + normalize + store ----------------------
            out_sb = outp.tile([128, OUTW], f32)
            for t in range(T):
                for r in range(RG):
                    o0 = (t * RG + r) * 8 * G64
                    if 'evacnorm' in SKIP:
                        nc.vector.tensor_copy(out_sb[:, o0:o0 + 8 * G64],
                                              psums[t][r][:, :])
                    else:
                        nc.scalar.activation(
                            out_sb[:, o0:o0 + 8 * G64], psums[t][r][:, :],
                            ACT.Identity,
                            bias=ns_bc[:, T + t:T + t + 1],
                            scale=ns_bc[:, t:t + 1])
                    nc.sync.dma_start(out_d[:, (t * RG + r) * 8 * G64:
                                            (t * RG + r + 1) * 8 * G64],
                                      out_sb[:, (t * RG + r) * 8 * G64:
                                             (t * RG + r + 1) * 8 * G64])

    nc.finalize()
    return nc, RG, OUTW


def kernel(feat, distances, src, dst, cutoffs, rbf_means, rbf_scaling,
           features_to_use, bn_weight, bn_bias):
    from concourse.bass_utils import run_bass_kernel_spmd

    cutoffs = np.asarray(cutoffs, dtype=np.float32)
    rbf_means = np.asarray(rbf_means, dtype=np.float32)
    rbf_scaling = np.asarray(rbf_scaling, dtype=np.float32)
    features_to_use = np.asarray(features_to_use, dtype=np.float32)
    bn_weight = np.asarray(bn_weight, dtype=np.float32)
    bn_bias = np.asarray(bn_bias, dtype=np.float32)
    assert np.all(cutoffs == cutoffs[0]), "kernel specialized to uniform cutoffs"
    assert np.all(rbf_scaling == rbf_scaling[0]), "specialized to uniform gamma"
    gam_val = float(rbf_scaling[0])
    co_val = float(cutoffs[0])
    need_mask = not bool(np.asarray(distances).max() < co_val)

    dts, mus, fss, written, NBW, offs, GT, ranks, J = _build_layout(
        feat, distances, src, dst, rbf_means)

    d_pad = float(max(cutoffs.max(), rbf_means.max()) + 100.0)
    dts[:, :, :, :][~np.broadcast_to(written[:, :, None, :], dts.shape)] = d_pad

    par = np.concatenate([rbf_means, rbf_scaling, cutoffs, features_to_use,
                          bn_weight, bn_bias]).astype(np.float32)[None, :]
    NP = par.shape[1]

    key = (GT, tuple(NBW), J, NP, gam_val, co_val, need_mask)
    if key not in _COMPILED:
        _COMPILED[key] = _build_graph(GT, NBW, offs, J, NP, gam_val, co_val,
                                      need_mask)
    nc, RG, OUTW = _COMPILED[key]

    in_maps = [
        {"dts": np.ascontiguousarray(dts[c].reshape(128, 8 * GT)),
         "mus": np.ascontiguousarray(mus[c]),
         "fss": np.ascontiguousarray(fss[c]),
         "par": par}
        for c in range(NCORES)
    ]
    trace = os.environ.get("BASS_PROF", "0") == "1"
    res = run_bass_kernel_spmd(nc, in_maps, core_ids=list(range(NCORES)),
                               trace=trace)
    global LAST_RESULT
    LAST_RESULT = res
    if trace and res.exec_time_ns is not None:
        print(f"HW exec time: {res.exec_time_ns} ns")
    outs = res.results

    # decode: out [128, OUTW] -> (p=(n32,s... wait p raw), cols (t, r, k, g))
    full = np.empty((N, T, K), dtype=np.float32)
    for c in range(NCORES):
        o = outs[c]["out"].reshape(NSTACK, BLK, T, RG, K, G64)
        # p = su*32 + n32  -> (su, n32); rank = (r*256 + su*64 + g)*32 + n32
        hvr = o.transpose(3, 0, 5, 1, 2, 4).reshape(-1, T, K)
        full[c * NPC:(c + 1) * NPC] = hvr[ranks[c]]
    return full
